# revision 6
# baseline (speedup 1.0000x reference)
"""Distributed Trainium2 kernel for the audio-visual contrastive loss.

Math (reference):
    a = l2norm(audio)  (B=32, Na=512, D=768)
    v = l2norm(visual) (B=32, Nv=256, D=768)
    token_sims[b,c,n,m] = (a[b,n] . v[c,m]) / T
    clip_sims = mean_n max_m token_sims          (B, B)
    loss = mean_b -0.5*(log_softmax(clip)[b,b] + log_softmax(clip.T)[b,b])

Distribution over 8 NeuronCores (XOR-slot peer exchange, no collective
on the critical path):
    - batch sharded 4 audio + 4 visual clips per core; inputs arrive as
      bf16 and are transposed d-major via the DMA crossbar (XBAR), then
      cast to fp8 on GpSimd.
    - audio rows are NOT pre-normalized: max_m is invariant to a positive
      per-row scale, so 1/||a_n|| is folded into the final column-sum
      matmul (lhsT = reciprocal norms instead of ones).
    - each core broadcasts its prepped visual shard to the 7 peers with
      XOR-relative remote DMA: slot k on core i holds the shard of core
      i^k, so slot 0 (own shard) is computable immediately while the
      exchange is in flight. Per-slot arrival is gated by remote
      semaphores attached post-scheduling (the Tile scheduler cannot see
      remote increments).
    - the (4,32) clip-sim blocks (columns in XOR-slot order) are
      AllGather'd (tiny) and every core computes the loss redundantly;
      the column logsumexp realigns the per-row-block XOR column
      permutation with 8 constant permutation matmuls.
"""

import sys

for _p in ("/opt/trn_rl_repo",):
    if _p not in sys.path:
        sys.path.insert(0, _p)

import numpy as np

import concourse.bacc as bacc
import concourse.mybir as mybir
import concourse.tile as tile
from concourse.tile_rust import add_dep_helper

N_CORES = 8
B = 32
NA = 512
NV = 256
D = 768
TEMPERATURE = 0.1
BL = B // N_CORES            # 4 clips per core
AROWS = BL * NA              # 2048 audio rows per core
VROWS = BL * NV              # 1024 visual rows per core
KD = D // 128                # 6 contraction chunks
KD2 = KD // 2                # 3 DoubleRow chunk-pairs
NT_A = AROWS // 128          # 16 audio row-tiles
NT_V = VROWS // 128          # 8 visual row-tiles
VSH = KD * VROWS             # 6144 cols: one visual shard, (g,k,c)-major
SCLV = 16.0                  # visual fp8 pre-scale (folded into rsqrt)

F32 = mybir.dt.float32
BF16 = mybir.dt.bfloat16
FP8 = mybir.dt.float8e4
AX = mybir.AxisListType
ALU = mybir.AluOpType
ACT = mybir.ActivationFunctionType

# driver logical->physical NC map: the Q7 XORs dtpb against its own
# PHYSICAL tpb; with dtpb = PMAP[d] the logical slot relation is
# "slot d holds the shard of core (rank ^ d)" (PMAP is an involution
# preserving bit 2, so D2D slot placement stays legal).
PMAP = (0, 1, 2, 3, 6, 7, 4, 5)


def _tail_consts():
    # diagonal mask on the gathered [32,32] slot-ordered matrix:
    # M[4i+b, c] = clip(audio 4i+b, visual 4(i^(c//4)) + c%4), so the
    # true diagonal sits at column c = b (slot 0 = own block).
    dmask = np.zeros((32, 32), dtype=np.float32)
    for r in range(32):
        dmask[r, r % 4] = 1.0
    # block-diagonal ones: bdones[r, i] = 1 iff r//4 == i  ([32, 8])
    bdones = np.zeros((32, 8), dtype=np.float32)
    for r in range(32):
        bdones[r, r // 4] = 1.0
    # XOR realignment perms: P_i[j, c] = 1 iff 4*(i ^ (j//4)) + j%4 == c
    pcat = np.zeros((32, 8 * 32), dtype=np.float32)
    for i in range(8):
        for j in range(32):
            c = 4 * (i ^ (j // 4)) + (j % 4)
            pcat[j, i * 32 + c] = 1.0
    return dmask, bdones, pcat


def build():
    nc = bacc.Bacc("TRN2", target_bir_lowering=False, debug=False,
                   num_devices=N_CORES)
    a_in = nc.declare_dram_parameter("audio", [AROWS, D], BF16, isOutput=False)
    v_in = nc.declare_dram_parameter("visual", [VROWS, D], BF16,
                                     isOutput=False)
    out = nc.declare_dram_parameter("out", [1, 1], F32, isOutput=True)
    dmask_np, bdones_np, pcat_np = _tail_consts()
    dmask_dram = nc.inline_tensor(dmask_np, name="dmask")
    bdones_dram = nc.inline_tensor(bdones_np, name="bdones")
    pcat_dram = nc.inline_tensor(pcat_np, name="pcat")
    ident_dram = nc.inline_tensor(np.eye(32, dtype=np.float32), name="ident")
    nones_dram = nc.inline_tensor(
        np.concatenate([np.ones((32, 1), dtype=np.float32),
                       -np.ones((32, 1), dtype=np.float32)], axis=1),
        name="nones")
    rg = [list(range(N_CORES))]

    lsem = nc.alloc_semaphore("rdma_lsem")
    rsems = [nc.alloc_semaphore(f"rdma_rs{k}") for k in range(1, N_CORES)]
    anchors = []

    with tile.TileContext(nc) as tc:
        with (
            tc.tile_pool(name="persist", bufs=1) as pp,
            tc.tile_pool(name="work", bufs=3) as wp,
            tc.tile_pool(name="ps", bufs=8, space="PSUM") as ps,
            tc.tile_pool(name="dram", bufs=1, space="DRAM") as dp,
        ):
            # ---- constants (sync ring, early) -----------------------------
            dmask = pp.tile([32, 32], F32, tag="dmask")
            nc.sync.dma_start(out=dmask[:], in_=dmask_dram[:])
            bdones = pp.tile([32, 8], F32, tag="bdones")
            nc.sync.dma_start(out=bdones[:], in_=bdones_dram[:])
            pcat = pp.tile([32, 8 * 32], F32, tag="pcat")
            nc.sync.dma_start(out=pcat[:], in_=pcat_dram[:])
            ident = pp.tile([32, 32], F32, tag="ident")
            nc.sync.dma_start(out=ident[:], in_=ident_dram[:])
            nones = pp.tile([32, 2], F32, tag="nones")
            nc.sync.dma_start(out=nones[:], in_=nones_dram[:])

            # ---- persistent tensors ---------------------------------------
            # vT: 8 XOR slots, each [128, VSH] fp8, (g, k, c)-major:
            #     col = d*VSH + g*3072 + k*512 + c
            vT = pp.tile([128, N_CORES * VSH], FP8, tag="vT", name="vT")
            # audio d-major bf16 staging + fp8, per-k2 ko-paired layout:
            #     col = k2*4096 + ko*2048 + t*128 + r
            aTb = pp.tile([128, KD2 * 4096], BF16, tag="aTb", name="aTb")
            aTf = pp.tile([128, KD2 * 4096], FP8, tag="aTf", name="aTf")
            # visual d-major bf16 staging (own shard, pre-cast):
            #     col = g*3072 + k*512 + c
            vTb = pp.tile([128, VSH], BF16, tag="vTb", name="vTb")
            mx = pp.tile([128, NT_A * 32], F32, tag="mx")
            rna = pp.tile([128, NT_A], F32, tag="rna")   # 1/||a_row||

            # warmup collective: absorbs first-collective staging latency
            wu_in = dp.tile([1, 32], F32, tag="wu_in", name="wu_in")
            wu_out = dp.tile([N_CORES, 32], F32, tag="wu_out", name="wu_out",
                             addr_space="Shared")
            wu_sb = pp.tile([1, 32], F32, tag="wu_sb")
            nc.gpsimd.memset(wu_sb[:], 0.0)
            nc.gpsimd.dma_start(out=wu_in[:], in_=wu_sb[:])
            nc.gpsimd.collective_compute(
                "AllGather", ALU.bypass, replica_groups=rg,
                ins=[wu_in[:, :].opt()], outs=[wu_out[:, :].opt()])

            # ---- visual prep: norm scale, transpose, cast, send -----------
            # per g-chunk of 4 tiles: row-major load -> ACT square+accum ->
            # sqrt -> recip -> ACT scaled bf16 copy -> 6 XBAR transposes
            vcasts = []
            for g in range(2):
                ssv = wp.tile([128, 4], F32, tag="ssv", name="ssv", bufs=2)
                raws, sq_h = [], None
                for j in range(4):
                    tv = g * 4 + j
                    raw = wp.tile([128, D], BF16, tag="vraw", name="vraw",
                                  bufs=8)
                    nc.scalar.dma_start(
                        out=raw[:], in_=v_in[tv * 128:(tv + 1) * 128, :])
                    sqs = wp.tile([128, D], BF16, tag="vsqs", name="vsqs",
                                  bufs=2)
                    nc.scalar.activation(sqs[:], raw[:], ACT.Square,
                                         accum_out=ssv[:, j:j + 1])
                    raws.append(raw)
                nrm = wp.tile([128, 4], F32, tag="vnrm", name="vnrm", bufs=2)
                nc.scalar.activation(nrm[:], ssv[:], ACT.Sqrt,
                                     scale=1.0 / (SCLV * SCLV))
                rsv = wp.tile([128, 4], F32, tag="vrsv", name="vrsv", bufs=2)
                nc.vector.reciprocal(rsv[:], nrm[:])
                for j in range(4):
                    vn = wp.tile([128, D], BF16, tag="vn", name="vn", bufs=4)
                    nc.scalar.activation(vn[:], raws[j][:], ACT.Copy,
                                         bias=0.0, scale=rsv[:, j:j + 1])
                    for k in range(KD):
                        nc.scalar.dma_start_transpose(
                            out=vTb[:, g * 3072 + k * 512 + j * 128:
                                    g * 3072 + k * 512 + (j + 1) * 128],
                            in_=vn[:, k * 128:(k + 1) * 128])
                # fp8 cast of this g-chunk into own slot 0
                vcasts.append(nc.gpsimd.tensor_copy(
                    vT[:, g * 3072:(g + 1) * 3072],
                    vTb[:, g * 3072:(g + 1) * 3072]))

            # ---- XOR-slot broadcast of the own shard ----------------------
            # two 3072B/partition sends per peer (payloads above 4KB per
            # partition do not survive the descriptor path)
            for d in range(1, N_CORES):
                rdests = [None] * 8
                rdests[d] = (0, PMAP[d])
                for g in range(2):
                    nc.gpsimd.remote_dma_broadcast(
                        out_ap=vT[:, d * VSH + g * 3072:
                                  d * VSH + (g + 1) * 3072],
                        in_ap=vT[:, g * 3072:(g + 1) * 3072],
                        remote_sem=rsems[d - 1], local_sem=lsem,
                        rdests=rdests)
            trig = nc.gpsimd.trigger_dma(count=None)

            # ---- audio prep: XBAR transposes straight from DRAM -----------
            a3 = a_in[:].rearrange("(t p) d -> t p d", p=128)
            atrans = [[] for _ in range(KD2)]
            for t in range(NT_A):
                for k in range(KD):
                    k2, ko = k // 2, k % 2
                    atrans[k2].append(nc.sync.dma_start_transpose(
                        out=aTb[:, k2 * 4096 + ko * 2048 + t * 128:
                                k2 * 4096 + ko * 2048 + (t + 1) * 128],
                        in_=a3[t, :, k * 128:(k + 1) * 128]))
            acasts = []
            for k2 in range(KD2):
                for q in range(4):
                    c0 = k2 * 4096 + q * 1024
                    acasts.append(nc.gpsimd.tensor_copy(
                        aTf[:, c0:c0 + 1024], aTb[:, c0:c0 + 1024]))

            # audio row norms (folded into the final column-sum matmul)
            for h in range(2):
                ssa = wp.tile([128, 8], F32, tag="ssa", name="ssa", bufs=2)
                for j in range(8):
                    t = h * 8 + j
                    raw = wp.tile([128, D], BF16, tag="araw", name="araw",
                                  bufs=8)
                    nc.scalar.dma_start(
                        out=raw[:], in_=a_in[t * 128:(t + 1) * 128, :])
                    sqs = wp.tile([128, D], BF16, tag="asqs", name="asqs",
                                  bufs=2)
                    nc.scalar.activation(sqs[:], raw[:], ACT.Square,
                                         accum_out=ssa[:, j:j + 1])
                nrm = wp.tile([128, 8], F32, tag="anrm", name="anrm", bufs=2)
                nc.scalar.activation(nrm[:], ssa[:], ACT.Sqrt)
                nc.vector.reciprocal(rna[:, h * 8:(h + 1) * 8], nrm[:])

            # ---- per-slot arrival anchors ---------------------------------
            # local nops carrying the remote-semaphore waits (attached
            # post-scheduling); gpsimd queue order: casts -> anchors.
            for d in range(1, N_CORES):
                a = nc.gpsimd.nop()
                add_dep_helper(a.ins, trig.ins, sync=False,
                               reason="anchor after trigger")
                for c in acasts + vcasts:
                    add_dep_helper(a.ins, c.ins, sync=False,
                                   reason="casts before anchors")
                anchors.append(a)

            # ---- main loop: S = aT.T @ vT (fp8 DoubleRow), rowmax ---------
            # mx col = t*32 + d*4 + g*2 + {0,1}
            aT3 = aTf[:].rearrange("p (k2 ko m) -> p k2 ko m", k2=KD2, ko=2)
            vT4 = vT[:].rearrange("p (d g k c) -> p d g k c",
                                  d=N_CORES, g=2, c=512)
            pclip = ps.tile([1, 512], F32, tag="ps", name="pclip")
            for d in range(N_CORES):
                for g in range(2):
                    for t in range(NT_A):
                        pss = ps.tile([128, 512], F32, tag="ps", name="mm")
                        for k2 in range(KD2):
                            mm = nc.tensor.matmul(
                                pss[:],
                                lhsT=aT3[:, k2, :, t * 128:(t + 1) * 128],
                                rhs=vT4[:, d, g, 2 * k2:2 * k2 + 2, :],
                                start=(k2 == 0), stop=(k2 == KD2 - 1),
                                perf_mode=mybir.MatmulPerfMode.DoubleRow)
                            if d > 0:
                                add_dep_helper(mm.ins, anchors[d - 1].ins,
                                               sync=True,
                                               reason="slot arrival")
                        mcol = t * 32 + d * 4 + g * 2
                        nc.vector.tensor_reduce(
                            out=mx[:, mcol:mcol + 2],
                            in_=pss[:].rearrange("p (j m) -> p j m", j=2),
                            axis=AX.X, op=ALU.max)
            # column sums weighted by 1/||a_row||: mean over n + norm fold
            for t in range(NT_A):
                nc.tensor.matmul(pclip[:, t * 32:(t + 1) * 32],
                                 lhsT=rna[:, t:t + 1],
                                 rhs=mx[:, t * 32:(t + 1) * 32],
                                 start=True, stop=True)

            # ---- clip block: reduce over nt, scale ------------------------
            # pclip col = (b*4 + nt)*32 + c ; want [1, b*32 + c]
            csum = wp.tile([1, 128], F32, tag="csum")
            nc.vector.tensor_reduce(
                out=csum[:],
                in_=pclip[:].rearrange("p (b nt c) -> p b c nt", b=4, nt=4),
                axis=AX.X, op=ALU.add)
            clip_blk = wp.tile([1, 128], F32, tag="clipblk")
            nc.scalar.mul(clip_blk[:], csum[:],
                          1.0 / (NA * TEMPERATURE * SCLV))

            # ---- gather the (4,32) clip blocks (tiny AllGather) -----------
            cb = dp.tile([1, 128], F32, tag="cb", name="cb")
            nc.sync.dma_start(out=cb[:], in_=clip_blk[:])
            call = dp.tile([N_CORES, 128], F32, tag="call", name="call",
                           addr_space="Shared")
            nc.gpsimd.collective_compute(
                "AllGather", ALU.bypass, replica_groups=rg,
                ins=[cb[:, :].opt()], outs=[call[:, :].opt()])

            # ---- final loss (computed redundantly on every core) ----------
            M = wp.tile([32, 32], F32, tag="M")
            nc.sync.dma_start(
                out=M[:], in_=call[:, :].rearrange("a (b c) -> (a b) c", b=4))

            # row logsumexp (permutation-invariant per row)
            E = wp.tile([32, 32], F32, tag="E")
            es = wp.tile([32, 1], F32, tag="es")
            nc.scalar.activation(E[:], M[:], ACT.Exp, accum_out=es[:])
            rowlse = wp.tile([32, 1], F32, tag="rowlse")
            nc.scalar.activation(rowlse[:], es[:], ACT.Ln)

            # diagonal (static slot-0 position)
            dsc = wp.tile([32, 32], F32, tag="dsc")
            nc.vector.tensor_mul(dsc[:], M[:], dmask[:])
            diag = wp.tile([32, 1], F32, tag="diag")
            nc.vector.reduce_sum(out=diag[:], in_=dsc[:], axis=AX.X)

            # column sums of exp with per-row-block XOR realignment:
            # RS[i, :] = sum of E rows in block i ; colsum = sum_i P_i @ RS_i
            psRS = ps.tile([8, 32], F32, tag="ps", name="psRS")
            nc.tensor.matmul(psRS[:], lhsT=bdones[:], rhs=E[:],
                             start=True, stop=True)
            RS = wp.tile([8, 32], F32, tag="RS")
            nc.vector.tensor_copy(RS[:], psRS[:])
            psRST = ps.tile([32, 8], F32, tag="ps", name="psRST")
            nc.tensor.matmul(psRST[:], lhsT=RS[:], rhs=ident[0:8, 0:8],
                             is_transpose=True)
            RST = wp.tile([32, 8], F32, tag="RST")
            nc.vector.tensor_copy(RST[:], psRST[:])
            pscol = ps.tile([32, 1], F32, tag="ps", name="pscol")
            for i in range(8):
                nc.tensor.matmul(pscol[:], lhsT=pcat[:, i * 32:(i + 1) * 32],
                                 rhs=RST[:, i:i + 1],
                                 start=(i == 0), stop=(i == 7))
            collse = wp.tile([32, 1], F32, tag="collse")
            nc.scalar.activation(collse[:], pscol[:], ACT.Ln)

            # loss = -(1/64) * [ sum_b (2 diag - rowlse) - sum_c collse ]
            lb = wp.tile([32, 1], F32, tag="lb")
            nc.vector.scalar_tensor_tensor(
                out=lb[:], in0=diag[:], scalar=2.0, in1=rowlse[:],
                op0=ALU.mult, op1=ALU.subtract)
            pl = ps.tile([1, 1], F32, tag="ps", name="pl")
            nc.tensor.matmul(pl[:], lhsT=lb[:], rhs=nones[:, 0:1],
                             start=True, stop=False)
            nc.tensor.matmul(pl[:], lhsT=collse[:], rhs=nones[:, 1:2],
                             start=False, stop=True)
            res = wp.tile([1, 1], F32, tag="res")
            nc.scalar.mul(res[:], pl[:], -1.0 / 64.0)
            od = nc.sync.dma_start(out=out[:], in_=res[:])

            # ---- semaphore hygiene for repeated executions ----------------
            # (lsem is never waited on with absolute thresholds, so it can
            # stay uncleared; the sim race detector also cannot see through
            # its DMA-completion updates.)
            for s in rsems:
                c = nc.gpsimd.sem_clear(s)
                add_dep_helper(c.ins, od.ins, sync=False,
                               reason="clear at end")
                for a in anchors:
                    add_dep_helper(c.ins, a.ins, sync=False,
                                   reason="clear after waits")

    # ---- post-scheduling: attach remote-semaphore waits -------------------
    # (2 increments per arriving send, 2 sends per peer)
    for d in range(1, N_CORES):
        anchors[d - 1].wait_op(rsems[d - 1], 4, "sem-ge")

    nc.finalize()
    return nc


_NC_CACHE = None


def kernel(audio_feats: np.ndarray, visual_feats: np.ndarray) -> np.ndarray:
    import ml_dtypes
    from concourse.bass_utils import run_bass_kernel_spmd

    global _NC_CACHE
    if _NC_CACHE is None:
        _NC_CACHE = build()
    nc = _NC_CACHE

    audio = np.asarray(audio_feats, dtype=np.float32).astype(
        ml_dtypes.bfloat16)
    visual = np.asarray(visual_feats, dtype=np.float32).astype(
        ml_dtypes.bfloat16)
    in_maps = []
    for i in range(N_CORES):
        in_maps.append({
            "audio": np.ascontiguousarray(
                audio[i * BL:(i + 1) * BL].reshape(AROWS, D)),
            "visual": np.ascontiguousarray(
                visual[i * BL:(i + 1) * BL].reshape(VROWS, D)),
        })
    res = run_bass_kernel_spmd(nc, in_maps, core_ids=list(range(N_CORES)))
    val = res.results[0]["out"][0, 0]
    return np.asarray(val, dtype=np.float32)


if __name__ == "__main__":
    rng = np.random.default_rng(0)
    a = rng.standard_normal((B, NA, D)).astype(np.float32)
    v = rng.standard_normal((B, NV, D)).astype(np.float32)
    print(kernel(a, v))


# revision 12
# speedup vs baseline: 1.0891x; 1.0891x over previous
"""Distributed Trainium2 kernel for the audio-visual contrastive loss.

Math (reference):
    a = l2norm(audio)  (B=32, Na=512, D=768)
    v = l2norm(visual) (B=32, Nv=256, D=768)
    token_sims[b,c,n,m] = (a[b,n] . v[c,m]) / T
    clip_sims = mean_n max_m token_sims          (B, B)
    loss = mean_b -0.5*(log_softmax(clip)[b,b] + log_softmax(clip.T)[b,b])

Distribution over 8 NeuronCores (XOR-slot peer exchange, no collective
on the critical path):
    - batch sharded 4 audio + 4 visual clips per core; inputs arrive as
      bf16 and are transposed d-major via the DMA crossbar (XBAR), then
      cast to fp8 on GpSimd.
    - audio rows are NOT pre-normalized: max_m is invariant to a positive
      per-row scale, so 1/||a_n|| is folded into the final column-sum
      matmul (lhsT = reciprocal norms instead of ones).
    - each core broadcasts its prepped visual shard to the 7 peers with
      XOR-relative remote DMA: slot k on core i holds the shard of core
      i^k, so slot 0 (own shard) is computable immediately while the
      exchange is in flight. Per-slot arrival is gated by remote
      semaphores attached post-scheduling (the Tile scheduler cannot see
      remote increments).
    - the (4,32) clip-sim blocks (columns in XOR-slot order) are
      AllGather'd (tiny) and every core computes the loss redundantly;
      the column logsumexp realigns the per-row-block XOR column
      permutation with 8 constant permutation matmuls.
"""

import sys

for _p in ("/opt/trn_rl_repo",):
    if _p not in sys.path:
        sys.path.insert(0, _p)

import numpy as np

import concourse.bacc as bacc
import concourse.mybir as mybir
import concourse.tile as tile
from concourse.tile_rust import add_dep_helper

N_CORES = 8
B = 32
NA = 512
NV = 256
D = 768
TEMPERATURE = 0.1
BL = B // N_CORES            # 4 clips per core
AROWS = BL * NA              # 2048 audio rows per core
VROWS = BL * NV              # 1024 visual rows per core
KD = D // 128                # 6 contraction chunks
KD2 = KD // 2                # 3 DoubleRow chunk-pairs
NT_A = AROWS // 128          # 16 audio row-tiles
NT_V = VROWS // 128          # 8 visual row-tiles
VSH = KD * VROWS             # 6144 cols: one visual shard, (g,k,c)-major
SCLV = 16.0                  # visual fp8 pre-scale (folded into rsqrt)

F32 = mybir.dt.float32
BF16 = mybir.dt.bfloat16
FP8 = mybir.dt.float8e4
AX = mybir.AxisListType
ALU = mybir.AluOpType
ACT = mybir.ActivationFunctionType

# driver logical->physical NC map: the Q7 XORs dtpb against its own
# PHYSICAL tpb; with dtpb = PMAP[d] the logical slot relation is
# "slot d holds the shard of core (rank ^ d)" (PMAP is an involution
# preserving bit 2, so D2D slot placement stays legal).
PMAP = (0, 1, 2, 3, 6, 7, 4, 5)


def _tail_consts():
    # diagonal mask on the gathered [32,32] slot-ordered matrix:
    # M[4i+b, c] = clip(audio 4i+b, visual 4(i^(c//4)) + c%4), so the
    # true diagonal sits at column c = b (slot 0 = own block).
    dmask = np.zeros((32, 32), dtype=np.float32)
    for r in range(32):
        dmask[r, r % 4] = 1.0
    # block-diagonal ones: bdones[r, i] = 1 iff r//4 == i  ([32, 8])
    bdones = np.zeros((32, 8), dtype=np.float32)
    for r in range(32):
        bdones[r, r // 4] = 1.0
    # XOR realignment perms: P_i[j, c] = 1 iff 4*(i ^ (j//4)) + j%4 == c
    pcat = np.zeros((32, 8 * 32), dtype=np.float32)
    for i in range(8):
        for j in range(32):
            c = 4 * (i ^ (j // 4)) + (j % 4)
            pcat[j, i * 32 + c] = 1.0
    return dmask, bdones, pcat


def build():
    nc = bacc.Bacc("TRN2", target_bir_lowering=False, debug=False,
                   num_devices=N_CORES)
    a_in = nc.declare_dram_parameter("audio", [AROWS, D], BF16, isOutput=False)
    v_in = nc.declare_dram_parameter("visual", [VROWS, D], BF16,
                                     isOutput=False)
    out = nc.declare_dram_parameter("out", [1, 1], F32, isOutput=True)
    dmask_np, bdones_np, pcat_np = _tail_consts()
    dmask_dram = nc.inline_tensor(dmask_np, name="dmask")
    bdones_dram = nc.inline_tensor(bdones_np, name="bdones")
    pcat_dram = nc.inline_tensor(pcat_np, name="pcat")
    ident_dram = nc.inline_tensor(np.eye(32, dtype=np.float32), name="ident")
    ident128_dram = nc.inline_tensor(np.eye(128, dtype=np.float32),
                                     name="ident128")
    nones_dram = nc.inline_tensor(
        np.concatenate([np.ones((32, 1), dtype=np.float32),
                       -np.ones((32, 1), dtype=np.float32)], axis=1),
        name="nones")
    rg = [list(range(N_CORES))]

    lsem = nc.alloc_semaphore("rdma_lsem")
    rsems = [nc.alloc_semaphore(f"rdma_rs{k}") for k in range(1, N_CORES)]
    anchors = []

    with tile.TileContext(nc) as tc:
        with (
            tc.tile_pool(name="persist", bufs=1) as pp,
            tc.tile_pool(name="work", bufs=3) as wp,
            tc.tile_pool(name="ps", bufs=8, space="PSUM") as ps,
            tc.tile_pool(name="dram", bufs=1, space="DRAM") as dp,
        ):
            # ---- constants (sync ring, early) -----------------------------
            dmask = pp.tile([32, 32], F32, tag="dmask")
            nc.sync.dma_start(out=dmask[:], in_=dmask_dram[:])
            bdones = pp.tile([32, 8], F32, tag="bdones")
            nc.sync.dma_start(out=bdones[:], in_=bdones_dram[:])
            pcat = pp.tile([32, 8 * 32], F32, tag="pcat")
            nc.sync.dma_start(out=pcat[:], in_=pcat_dram[:])
            ident = pp.tile([32, 32], F32, tag="ident")
            nc.sync.dma_start(out=ident[:], in_=ident_dram[:])
            nones = pp.tile([32, 2], F32, tag="nones")
            nc.sync.dma_start(out=nones[:], in_=nones_dram[:])

            # ---- persistent tensors ---------------------------------------
            # vT: 8 XOR slots, each [128, VSH] fp8, (g, k, c)-major:
            #     col = d*VSH + g*3072 + k*512 + c
            vT = pp.tile([128, N_CORES * VSH], FP8, tag="vT", name="vT")
            # audio d-major fp8, per-k2 ko-paired layout:
            #     col = k2*4096 + ko*2048 + t*128 + r
            aTf = pp.tile([128, KD2 * 4096], FP8, tag="aTf", name="aTf")
            mx = pp.tile([128, NT_A * 32], F32, tag="mx")
            rna = pp.tile([128, NT_A], F32, tag="rna")   # 1/||a_row||
            # bf16 identity: stationary operand of plain-matmul transposes
            # (X.T = X.T @ I at ~2x the rate of transpose-mode)
            ident128 = pp.tile([128, 128], F32, tag="ident128")
            nc.sync.dma_start(out=ident128[:], in_=ident128_dram[:])
            identb = pp.tile([128, 128], BF16, tag="identb")
            nc.scalar.copy(identb[:], ident128[:])

            # warmup collective: absorbs first-collective staging latency
            wu_in = dp.tile([1, 32], F32, tag="wu_in", name="wu_in")
            wu_out = dp.tile([N_CORES, 32], F32, tag="wu_out", name="wu_out",
                             addr_space="Shared")
            wu_sb = pp.tile([1, 32], F32, tag="wu_sb")
            nc.gpsimd.memset(wu_sb[:], 0.0)
            nc.gpsimd.dma_start(out=wu_in[:], in_=wu_sb[:])
            nc.gpsimd.collective_compute(
                "AllGather", ALU.bypass, replica_groups=rg,
                ins=[wu_in[:, :].opt()], outs=[wu_out[:, :].opt()])

            # ---- visual prep: norm scale, transpose, cast, send -----------
            # per g-chunk of 4 tiles: row-major load -> ACT square+accum ->
            # sqrt -> recip -> ACT scaled bf16 copy -> 6 XBAR transposes
            vcasts = []
            for g in range(2):
                ssv = wp.tile([128, 4], F32, tag="ssv", name="ssv", bufs=2)
                raws, sq_h = [], None
                for j in range(4):
                    tv = g * 4 + j
                    raw = wp.tile([128, D], BF16, tag="vraw", name="vraw",
                                  bufs=8)
                    nc.scalar.dma_start(
                        out=raw[:], in_=v_in[tv * 128:(tv + 1) * 128, :])
                    sqs = wp.tile([128, D], BF16, tag="vsqs", name="vsqs",
                                  bufs=2)
                    nc.scalar.activation(sqs[:], raw[:], ACT.Square,
                                         accum_out=ssv[:, j:j + 1])
                    raws.append(raw)
                nrm = wp.tile([128, 4], F32, tag="vnrm", name="vnrm", bufs=2)
                nc.scalar.activation(nrm[:], ssv[:], ACT.Sqrt,
                                     scale=1.0 / (SCLV * SCLV))
                rsv = wp.tile([128, 4], F32, tag="vrsv", name="vrsv", bufs=2)
                nc.vector.reciprocal(rsv[:], nrm[:])
                for j in range(4):
                    vn = wp.tile([128, D], BF16, tag="vn", name="vn", bufs=4)
                    nc.scalar.activation(vn[:], raws[j][:], ACT.Copy,
                                         bias=0.0, scale=rsv[:, j:j + 1])
                    for k in range(KD):
                        pt = ps.tile([128, 128], F32, tag="ps", name="vpt")
                        nc.tensor.matmul(pt[:],
                                         lhsT=vn[:, k * 128:(k + 1) * 128],
                                         rhs=identb[:], start=True, stop=True)
                        c0 = g * 3072 + k * 512 + j * 128
                        vcasts.append(nc.scalar.copy(
                            vT[:, c0:c0 + 128], pt[:]))

            # ---- XOR-slot broadcast of the own shard ----------------------
            # two 3072B/partition sends per peer (payloads above 4KB per
            # partition do not survive the descriptor path)
            for d in range(1, N_CORES):
                rdests = [None] * 8
                rdests[d] = (0, PMAP[d])
                for g in range(2):
                    nc.gpsimd.remote_dma_broadcast(
                        out_ap=vT[:, d * VSH + g * 3072:
                                  d * VSH + (g + 1) * 3072],
                        in_ap=vT[:, g * 3072:(g + 1) * 3072],
                        remote_sem=rsems[d - 1], local_sem=lsem,
                        rdests=rdests)
            trig = nc.gpsimd.trigger_dma(count=None)

            # ---- audio prep: row-major load, PE transpose, fp8 copy -------
            # (audio is NOT normalized here: 1/||a_row|| is applied by the
            # final column-sum matmul, so no scaled-cast pass is needed)
            acasts = []
            for h in range(2):
                ssa = wp.tile([128, 8], F32, tag="ssa", name="ssa", bufs=2)
                for j in range(8):
                    t = h * 8 + j
                    raw = wp.tile([128, D], BF16, tag="araw", name="araw",
                                  bufs=8)
                    nc.scalar.dma_start(
                        out=raw[:], in_=a_in[t * 128:(t + 1) * 128, :])
                    for k in range(KD):
                        k2, ko = k // 2, k % 2
                        pt = ps.tile([128, 128], F32, tag="ps", name="apt")
                        nc.tensor.matmul(pt[:],
                                         lhsT=raw[:, k * 128:(k + 1) * 128],
                                         rhs=identb[:], start=True, stop=True)
                        c0 = k2 * 4096 + ko * 2048 + t * 128
                        acasts.append(nc.vector.tensor_copy(
                            aTf[:, c0:c0 + 128], pt[:]))
                    sqs = wp.tile([128, D], BF16, tag="asqs", name="asqs",
                                  bufs=2)
                    nc.scalar.activation(sqs[:], raw[:], ACT.Square,
                                         accum_out=ssa[:, j:j + 1])
                nrm = wp.tile([128, 8], F32, tag="anrm", name="anrm", bufs=2)
                nc.scalar.activation(nrm[:], ssa[:], ACT.Sqrt)
                nc.vector.reciprocal(rna[:, h * 8:(h + 1) * 8], nrm[:])

            # ---- per-slot arrival anchors ---------------------------------
            # local nops carrying the remote-semaphore waits (attached
            # post-scheduling)
            for d in range(1, N_CORES):
                a = nc.gpsimd.nop()
                add_dep_helper(a.ins, trig.ins, sync=False,
                               reason="anchor after trigger")
                anchors.append(a)

            # ---- main loop: S = aT.T @ vT (fp8 DoubleRow), rowmax ---------
            # mx col = t*32 + d*4 + g*2 + {0,1}
            aT3 = aTf[:].rearrange("p (k2 ko m) -> p k2 ko m", k2=KD2, ko=2)
            vT4 = vT[:].rearrange("p (d g k c) -> p d g k c",
                                  d=N_CORES, g=2, c=512)
            pclip = ps.tile([1, 512], F32, tag="ps", name="pclip")
            for d in range(N_CORES):
                for g in range(2):
                    for t in range(NT_A):
                        pss = ps.tile([128, 512], F32, tag="ps", name="mm")
                        for k2 in range(KD2):
                            mm = nc.tensor.matmul(
                                pss[:],
                                lhsT=aT3[:, k2, :, t * 128:(t + 1) * 128],
                                rhs=vT4[:, d, g, 2 * k2:2 * k2 + 2, :],
                                start=(k2 == 0), stop=(k2 == KD2 - 1),
                                perf_mode=mybir.MatmulPerfMode.DoubleRow)
                            if d > 0:
                                add_dep_helper(mm.ins, anchors[d - 1].ins,
                                               sync=True,
                                               reason="slot arrival")
                        mcol = t * 32 + d * 4 + g * 2
                        nc.vector.tensor_reduce(
                            out=mx[:, mcol:mcol + 2],
                            in_=pss[:].rearrange("p (j m) -> p j m", j=2),
                            axis=AX.X, op=ALU.max)
            # column sums weighted by 1/||a_row||: mean over n + norm fold
            for t in range(NT_A):
                nc.tensor.matmul(pclip[:, t * 32:(t + 1) * 32],
                                 lhsT=rna[:, t:t + 1],
                                 rhs=mx[:, t * 32:(t + 1) * 32],
                                 start=True, stop=True)

            # ---- clip block: reduce over nt, scale ------------------------
            # pclip col = (b*4 + nt)*32 + c ; want [1, b*32 + c]
            csum = wp.tile([1, 128], F32, tag="csum")
            nc.vector.tensor_reduce(
                out=csum[:],
                in_=pclip[:].rearrange("p (b nt c) -> p b c nt", b=4, nt=4),
                axis=AX.X, op=ALU.add)
            clip_blk = wp.tile([1, 128], F32, tag="clipblk")
            nc.scalar.mul(clip_blk[:], csum[:],
                          1.0 / (NA * TEMPERATURE * SCLV))

            # ---- gather the (4,32) clip blocks (tiny AllGather) -----------
            cb = dp.tile([1, 128], F32, tag="cb", name="cb")
            nc.sync.dma_start(out=cb[:], in_=clip_blk[:])
            call = dp.tile([N_CORES, 128], F32, tag="call", name="call",
                           addr_space="Shared")
            nc.gpsimd.collective_compute(
                "AllGather", ALU.bypass, replica_groups=rg,
                ins=[cb[:, :].opt()], outs=[call[:, :].opt()])

            # ---- final loss (computed redundantly on every core) ----------
            M = wp.tile([32, 32], F32, tag="M")
            nc.sync.dma_start(
                out=M[:], in_=call[:, :].rearrange("a (b c) -> (a b) c", b=4))

            # row logsumexp (permutation-invariant per row)
            E = wp.tile([32, 32], F32, tag="E")
            es = wp.tile([32, 1], F32, tag="es")
            nc.scalar.activation(E[:], M[:], ACT.Exp, accum_out=es[:])
            rowlse = wp.tile([32, 1], F32, tag="rowlse")
            nc.scalar.activation(rowlse[:], es[:], ACT.Ln)

            # diagonal (static slot-0 position)
            dsc = wp.tile([32, 32], F32, tag="dsc")
            nc.vector.tensor_mul(dsc[:], M[:], dmask[:])
            diag = wp.tile([32, 1], F32, tag="diag")
            nc.vector.reduce_sum(out=diag[:], in_=dsc[:], axis=AX.X)

            # column sums of exp with per-row-block XOR realignment:
            # RS[i, :] = sum of E rows in block i ; colsum = sum_i P_i @ RS_i
            psRS = ps.tile([8, 32], F32, tag="ps", name="psRS")
            nc.tensor.matmul(psRS[:], lhsT=bdones[:], rhs=E[:],
                             start=True, stop=True)
            RS = wp.tile([8, 32], F32, tag="RS")
            nc.vector.tensor_copy(RS[:], psRS[:])
            psRST = ps.tile([32, 8], F32, tag="ps", name="psRST")
            nc.tensor.matmul(psRST[:], lhsT=RS[:], rhs=ident[0:8, 0:8],
                             is_transpose=True)
            RST = wp.tile([32, 8], F32, tag="RST")
            nc.vector.tensor_copy(RST[:], psRST[:])
            pscol = ps.tile([32, 1], F32, tag="ps", name="pscol")
            for i in range(8):
                nc.tensor.matmul(pscol[:], lhsT=pcat[:, i * 32:(i + 1) * 32],
                                 rhs=RST[:, i:i + 1],
                                 start=(i == 0), stop=(i == 7))
            collse = wp.tile([32, 1], F32, tag="collse")
            nc.scalar.activation(collse[:], pscol[:], ACT.Ln)

            # loss = -(1/64) * [ sum_b (2 diag - rowlse) - sum_c collse ]
            lb = wp.tile([32, 1], F32, tag="lb")
            nc.vector.scalar_tensor_tensor(
                out=lb[:], in0=diag[:], scalar=2.0, in1=rowlse[:],
                op0=ALU.mult, op1=ALU.subtract)
            pl = ps.tile([1, 1], F32, tag="ps", name="pl")
            nc.tensor.matmul(pl[:], lhsT=lb[:], rhs=nones[:, 0:1],
                             start=True, stop=False)
            nc.tensor.matmul(pl[:], lhsT=collse[:], rhs=nones[:, 1:2],
                             start=False, stop=True)
            res = wp.tile([1, 1], F32, tag="res")
            nc.scalar.mul(res[:], pl[:], -1.0 / 64.0)
            od = nc.sync.dma_start(out=out[:], in_=res[:])

            # ---- semaphore hygiene for repeated executions ----------------
            # (lsem is never waited on with absolute thresholds, so it can
            # stay uncleared; the sim race detector also cannot see through
            # its DMA-completion updates.)
            for s in rsems:
                c = nc.gpsimd.sem_clear(s)
                add_dep_helper(c.ins, od.ins, sync=False,
                               reason="clear at end")
                for a in anchors:
                    add_dep_helper(c.ins, a.ins, sync=False,
                                   reason="clear after waits")

    # ---- post-scheduling: attach remote-semaphore waits -------------------
    # (2 increments per arriving send, 2 sends per peer)
    for d in range(1, N_CORES):
        anchors[d - 1].wait_op(rsems[d - 1], 4, "sem-ge")

    nc.finalize()
    return nc


_NC_CACHE = None


def kernel(audio_feats: np.ndarray, visual_feats: np.ndarray) -> np.ndarray:
    import ml_dtypes
    from concourse.bass_utils import run_bass_kernel_spmd

    global _NC_CACHE
    if _NC_CACHE is None:
        _NC_CACHE = build()
    nc = _NC_CACHE

    audio = np.asarray(audio_feats, dtype=np.float32).astype(
        ml_dtypes.bfloat16)
    visual = np.asarray(visual_feats, dtype=np.float32).astype(
        ml_dtypes.bfloat16)
    in_maps = []
    for i in range(N_CORES):
        in_maps.append({
            "audio": np.ascontiguousarray(
                audio[i * BL:(i + 1) * BL].reshape(AROWS, D)),
            "visual": np.ascontiguousarray(
                visual[i * BL:(i + 1) * BL].reshape(VROWS, D)),
        })
    res = run_bass_kernel_spmd(nc, in_maps, core_ids=list(range(N_CORES)))
    val = res.results[0]["out"][0, 0]
    return np.asarray(val, dtype=np.float32)


if __name__ == "__main__":
    rng = np.random.default_rng(0)
    a = rng.standard_normal((B, NA, D)).astype(np.float32)
    v = rng.standard_normal((B, NV, D)).astype(np.float32)
    print(kernel(a, v))


# revision 16
# speedup vs baseline: 1.8732x; 1.7200x over previous
"""Distributed Trainium2 kernel for the audio-visual contrastive loss.

Math (reference):
    a = l2norm(audio)  (B=32, Na=512, D=768)
    v = l2norm(visual) (B=32, Nv=256, D=768)
    token_sims[b,c,n,m] = (a[b,n] . v[c,m]) / T
    clip_sims = mean_n max_m token_sims          (B, B)
    loss = mean_b -0.5*(log_softmax(clip)[b,b] + log_softmax(clip.T)[b,b])

Distribution over 8 NeuronCores (XOR-slot peer exchange, no collective
on the critical path):
    - batch sharded 4 audio + 4 visual clips per core; inputs arrive as
      bf16 and are transposed d-major via the DMA crossbar (XBAR), then
      cast to fp8 on GpSimd.
    - audio rows are NOT pre-normalized: max_m is invariant to a positive
      per-row scale, so 1/||a_n|| is folded into the final column-sum
      matmul (lhsT = reciprocal norms instead of ones).
    - each core broadcasts its prepped visual shard to the 7 peers with
      XOR-relative remote DMA: slot k on core i holds the shard of core
      i^k, so slot 0 (own shard) is computable immediately while the
      exchange is in flight. Per-slot arrival is gated by remote
      semaphores attached post-scheduling (the Tile scheduler cannot see
      remote increments).
    - the (4,32) clip-sim blocks (columns in XOR-slot order) are
      AllGather'd (tiny) and every core computes the loss redundantly;
      the column logsumexp realigns the per-row-block XOR column
      permutation with 8 constant permutation matmuls.
"""

import sys

for _p in ("/opt/trn_rl_repo",):
    if _p not in sys.path:
        sys.path.insert(0, _p)

import numpy as np

import concourse.bacc as bacc
import concourse.mybir as mybir
import concourse.tile as tile
from concourse.tile_rust import add_dep_helper

N_CORES = 8
B = 32
NA = 512
NV = 256
D = 768
TEMPERATURE = 0.1
BL = B // N_CORES            # 4 clips per core
AROWS = BL * NA              # 2048 audio rows per core
VROWS = BL * NV              # 1024 visual rows per core
KD = D // 128                # 6 contraction chunks
KD2 = KD // 2                # 3 DoubleRow chunk-pairs
NT_A = AROWS // 128          # 16 audio row-tiles
NT_V = VROWS // 128          # 8 visual row-tiles
VSH = KD * VROWS             # 6144 cols: one visual shard, (g,k,c)-major
SCLV = 16.0                  # visual fp8 pre-scale (folded into rsqrt)

F32 = mybir.dt.float32
BF16 = mybir.dt.bfloat16
FP8 = mybir.dt.float8e4
AX = mybir.AxisListType
ALU = mybir.AluOpType
ACT = mybir.ActivationFunctionType

# driver logical->physical NC map: the Q7 XORs dtpb against its own
# PHYSICAL tpb; with dtpb = PMAP[d] the logical slot relation is
# "slot d holds the shard of core (rank ^ d)" (PMAP is an involution
# preserving bit 2, so D2D slot placement stays legal).
PMAP = (0, 1, 2, 3, 6, 7, 4, 5)


def _tail_consts():
    # diagonal mask on the gathered [32,32] slot-ordered matrix:
    # M[4i+b, c] = clip(audio 4i+b, visual 4(i^(c//4)) + c%4), so the
    # true diagonal sits at column c = b (slot 0 = own block).
    dmask = np.zeros((32, 32), dtype=np.float32)
    for r in range(32):
        dmask[r, r % 4] = 1.0
    # block-diagonal ones: bdones[r, i] = 1 iff r//4 == i  ([32, 8])
    bdones = np.zeros((32, 8), dtype=np.float32)
    for r in range(32):
        bdones[r, r // 4] = 1.0
    # XOR realignment perms: P_i[j, c] = 1 iff 4*(i ^ (j//4)) + j%4 == c
    pcat = np.zeros((32, 8 * 32), dtype=np.float32)
    for i in range(8):
        for j in range(32):
            c = 4 * (i ^ (j // 4)) + (j % 4)
            pcat[j, i * 32 + c] = 1.0
    return dmask, bdones, pcat


def build():
    nc = bacc.Bacc("TRN2", target_bir_lowering=False, debug=False,
                   num_devices=N_CORES)
    a_in = nc.declare_dram_parameter("audio", [AROWS, D], BF16, isOutput=False)
    v_in = nc.declare_dram_parameter("visual", [VROWS, D], BF16,
                                     isOutput=False)
    out = nc.declare_dram_parameter("out", [1, 1], F32, isOutput=True)
    dmask_np, bdones_np, pcat_np = _tail_consts()
    dmask_dram = nc.inline_tensor(dmask_np, name="dmask")
    bdones_dram = nc.inline_tensor(bdones_np, name="bdones")
    pcat_dram = nc.inline_tensor(pcat_np, name="pcat")
    ident_dram = nc.inline_tensor(np.eye(32, dtype=np.float32), name="ident")
    ident128_dram = nc.inline_tensor(np.eye(128, dtype=np.float32),
                                     name="ident128")
    nones_dram = nc.inline_tensor(
        np.concatenate([np.ones((32, 1), dtype=np.float32),
                       -np.ones((32, 1), dtype=np.float32)], axis=1),
        name="nones")
    rg = [list(range(N_CORES))]

    lsem = nc.alloc_semaphore("rdma_lsem")
    rsems = [nc.alloc_semaphore(f"rdma_rs{k}") for k in range(1, N_CORES)]
    anchors = []

    with tile.TileContext(nc) as tc:
        with (
            tc.tile_pool(name="persist", bufs=1) as pp,
            tc.tile_pool(name="work", bufs=3) as wp,
            tc.tile_pool(name="ps", bufs=8, space="PSUM") as ps,
            tc.tile_pool(name="dram", bufs=1, space="DRAM") as dp,
        ):
            # ---- constants (sync ring, early) -----------------------------
            dmask = pp.tile([32, 32], F32, tag="dmask")
            nc.sync.dma_start(out=dmask[:], in_=dmask_dram[:])
            bdones = pp.tile([32, 8], F32, tag="bdones")
            nc.sync.dma_start(out=bdones[:], in_=bdones_dram[:])
            pcat = pp.tile([32, 8 * 32], F32, tag="pcat")
            nc.sync.dma_start(out=pcat[:], in_=pcat_dram[:])
            ident = pp.tile([32, 32], F32, tag="ident")
            nc.sync.dma_start(out=ident[:], in_=ident_dram[:])
            nones = pp.tile([32, 2], F32, tag="nones")
            nc.sync.dma_start(out=nones[:], in_=nones_dram[:])

            # ---- persistent tensors ---------------------------------------
            # vT: 8 XOR slots, each [128, VSH] fp8, (g, k, c)-major:
            #     col = d*VSH + g*3072 + k*512 + c
            vT = pp.tile([128, N_CORES * VSH], FP8, tag="vT", name="vT")
            # audio d-major fp8, per-k2 ko-paired layout:
            #     col = k2*4096 + ko*2048 + t*128 + r
            aTf = pp.tile([128, KD2 * 4096], FP8, tag="aTf", name="aTf")
            mx = pp.tile([128, NT_A * 32], F32, tag="mx")
            rna = pp.tile([128, NT_A], F32, tag="rna")   # 1/||a_row||
            # bf16 identity: stationary operand of plain-matmul transposes
            # (X.T = X.T @ I at ~2x the rate of transpose-mode)
            ident128 = pp.tile([128, 128], F32, tag="ident128")
            nc.sync.dma_start(out=ident128[:], in_=ident128_dram[:])
            identb = pp.tile([128, 128], BF16, tag="identb")
            nc.scalar.copy(identb[:], ident128[:])

            # warmup collective: absorbs first-collective staging latency
            wu_in = dp.tile([1, 32], F32, tag="wu_in", name="wu_in")
            wu_out = dp.tile([N_CORES, 32], F32, tag="wu_out", name="wu_out",
                             addr_space="Shared")
            wu_sb = pp.tile([1, 32], F32, tag="wu_sb")
            nc.gpsimd.memset(wu_sb[:], 0.0)
            nc.gpsimd.dma_start(out=wu_in[:], in_=wu_sb[:])
            nc.gpsimd.collective_compute(
                "AllGather", ALU.bypass, replica_groups=rg,
                ins=[wu_in[:, :].opt()], outs=[wu_out[:, :].opt()])

            # ---- visual prep: norm scale, transpose, cast, send -----------
            # per g-chunk of 4 tiles: row-major load -> ACT square+accum ->
            # sqrt -> recip -> ACT scaled bf16 copy -> 6 XBAR transposes
            vcasts = []
            for g in range(2):
                ssv = wp.tile([128, 4], F32, tag="ssv", name="ssv", bufs=2)
                raws = []
                for j in range(4):
                    tv = g * 4 + j
                    raw = wp.tile([128, D], BF16, tag="vraw", name="vraw",
                                  bufs=8)
                    nc.sync.dma_start(
                        out=raw[:], in_=v_in[tv * 128:(tv + 1) * 128, :])
                    sqs = wp.tile([128, D], BF16, tag="vsqs", name="vsqs",
                                  bufs=2)
                    nc.scalar.activation(sqs[:], raw[:], ACT.Square,
                                         accum_out=ssv[:, j:j + 1])
                    raws.append(raw)
                nrm = wp.tile([128, 4], F32, tag="vnrm", name="vnrm", bufs=2)
                nc.scalar.activation(nrm[:], ssv[:], ACT.Sqrt,
                                     scale=1.0 / (SCLV * SCLV))
                rsv = wp.tile([128, 4], F32, tag="vrsv", name="vrsv", bufs=2)
                nc.vector.reciprocal(rsv[:], nrm[:])
                for j in range(4):
                    vn = wp.tile([128, D], BF16, tag="vn", name="vn", bufs=4)
                    nc.scalar.activation(vn[:], raws[j][:], ACT.Copy,
                                         bias=0.0, scale=rsv[:, j:j + 1])
                    for k in range(KD):
                        pt = ps.tile([128, 128], F32, tag="ps", name="vpt")
                        nc.tensor.matmul(pt[:],
                                         lhsT=vn[:, k * 128:(k + 1) * 128],
                                         rhs=identb[:], start=True, stop=True)
                        c0 = g * 3072 + k * 512 + j * 128
                        if k % 2:
                            vcasts.append(nc.scalar.copy(
                                vT[:, c0:c0 + 128], pt[:]))
                        else:
                            vcasts.append(nc.vector.tensor_copy(
                                vT[:, c0:c0 + 128], pt[:]))

            # ---- XOR-slot broadcast of the own shard ----------------------
            # two 3072B/partition sends per peer (payloads above 4KB per
            # partition do not survive the descriptor path)
            for d in range(1, N_CORES):
                rdests = [None] * 8
                rdests[d] = (0, PMAP[d])
                for g in range(2):
                    nc.gpsimd.remote_dma_broadcast(
                        out_ap=vT[:, d * VSH + g * 3072:
                                  d * VSH + (g + 1) * 3072],
                        in_ap=vT[:, g * 3072:(g + 1) * 3072],
                        remote_sem=rsems[d - 1], local_sem=lsem,
                        rdests=rdests)
            trig = nc.gpsimd.trigger_dma(count=None)

            # ---- audio prep: row-major load, PE transpose, fp8 copy -------
            # (audio is NOT normalized here: 1/||a_row|| is applied by the
            # final column-sum matmul, so no scaled-cast pass is needed)
            acasts = []
            for h in range(2):
                ssa = wp.tile([128, 8], F32, tag="ssa", name="ssa", bufs=2)
                for j in range(8):
                    t = h * 8 + j
                    raw = wp.tile([128, D], BF16, tag="araw", name="araw",
                                  bufs=8)
                    nc.sync.dma_start(
                        out=raw[:], in_=a_in[t * 128:(t + 1) * 128, :])
                    for k in range(KD):
                        k2, ko = k // 2, k % 2
                        pt = ps.tile([128, 128], F32, tag="ps", name="apt")
                        nc.tensor.matmul(pt[:],
                                         lhsT=raw[:, k * 128:(k + 1) * 128],
                                         rhs=identb[:], start=True, stop=True)
                        c0 = k2 * 4096 + ko * 2048 + t * 128
                        acasts.append(nc.vector.tensor_copy(
                            aTf[:, c0:c0 + 128], pt[:]))
                    sqs = wp.tile([128, D], BF16, tag="asqs", name="asqs",
                                  bufs=2)
                    nc.scalar.activation(sqs[:], raw[:], ACT.Square,
                                         accum_out=ssa[:, j:j + 1])
                nrm = wp.tile([128, 8], F32, tag="anrm", name="anrm", bufs=2)
                nc.scalar.activation(nrm[:], ssa[:], ACT.Sqrt)
                nc.vector.reciprocal(rna[:, h * 8:(h + 1) * 8], nrm[:])

            # ---- per-slot arrival anchors ---------------------------------
            # local nops carrying the remote-semaphore waits (attached
            # post-scheduling)
            for d in range(1, N_CORES):
                a = nc.gpsimd.nop()
                add_dep_helper(a.ins, trig.ins, sync=False,
                               reason="anchor after trigger")
                anchors.append(a)

            # ---- main loop: S = aT.T @ vT (fp8 DoubleRow), rowmax ---------
            # mx col = t*32 + d*4 + g*2 + {0,1}
            aT3 = aTf[:].rearrange("p (k2 ko m) -> p k2 ko m", k2=KD2, ko=2)
            vT4 = vT[:].rearrange("p (d g k c) -> p d g k c",
                                  d=N_CORES, g=2, c=512)
            pclip = ps.tile([1, 512], F32, tag="ps", name="pclip")
            for d in range(N_CORES):
                for g in range(2):
                    for t in range(NT_A):
                        pss = ps.tile([128, 512], F32, tag="ps", name="mm")
                        for k2 in range(KD2):
                            mm = nc.tensor.matmul(
                                pss[:],
                                lhsT=aT3[:, k2, :, t * 128:(t + 1) * 128],
                                rhs=vT4[:, d, g, 2 * k2:2 * k2 + 2, :],
                                start=(k2 == 0), stop=(k2 == KD2 - 1),
                                perf_mode=mybir.MatmulPerfMode.DoubleRow)
                            if d > 0:
                                add_dep_helper(mm.ins, anchors[d - 1].ins,
                                               sync=True,
                                               reason="slot arrival")
                        mcol = t * 32 + d * 4 + g * 2
                        nc.vector.tensor_reduce(
                            out=mx[:, mcol:mcol + 2],
                            in_=pss[:].rearrange("p (j m) -> p j m", j=2),
                            axis=AX.X, op=ALU.max)
            # column sums weighted by 1/||a_row||: mean over n + norm fold
            for t in range(NT_A):
                nc.tensor.matmul(pclip[:, t * 32:(t + 1) * 32],
                                 lhsT=rna[:, t:t + 1],
                                 rhs=mx[:, t * 32:(t + 1) * 32],
                                 start=True, stop=True)

            # ---- clip block: reduce over nt, scale ------------------------
            # pclip col = (b*4 + nt)*32 + c ; want [1, b*32 + c]
            csum = wp.tile([1, 128], F32, tag="csum")
            nc.vector.tensor_reduce(
                out=csum[:],
                in_=pclip[:].rearrange("p (b nt c) -> p b c nt", b=4, nt=4),
                axis=AX.X, op=ALU.add)
            clip_blk = wp.tile([1, 128], F32, tag="clipblk")
            nc.scalar.mul(clip_blk[:], csum[:],
                          1.0 / (NA * TEMPERATURE * SCLV))

            # ---- gather the (4,32) clip blocks (tiny AllGather) -----------
            cb = dp.tile([1, 128], F32, tag="cb", name="cb")
            nc.sync.dma_start(out=cb[:], in_=clip_blk[:])
            call = dp.tile([N_CORES, 128], F32, tag="call", name="call",
                           addr_space="Shared")
            nc.gpsimd.collective_compute(
                "AllGather", ALU.bypass, replica_groups=rg,
                ins=[cb[:, :].opt()], outs=[call[:, :].opt()])

            # ---- final loss (computed redundantly on every core) ----------
            M = wp.tile([32, 32], F32, tag="M")
            nc.sync.dma_start(
                out=M[:], in_=call[:, :].rearrange("a (b c) -> (a b) c", b=4))

            # row logsumexp (permutation-invariant per row)
            E = wp.tile([32, 32], F32, tag="E")
            es = wp.tile([32, 1], F32, tag="es")
            nc.scalar.activation(E[:], M[:], ACT.Exp, accum_out=es[:])
            rowlse = wp.tile([32, 1], F32, tag="rowlse")
            nc.scalar.activation(rowlse[:], es[:], ACT.Ln)

            # diagonal (static slot-0 position)
            dsc = wp.tile([32, 32], F32, tag="dsc")
            nc.vector.tensor_mul(dsc[:], M[:], dmask[:])
            diag = wp.tile([32, 1], F32, tag="diag")
            nc.vector.reduce_sum(out=diag[:], in_=dsc[:], axis=AX.X)

            # column sums of exp with per-row-block XOR realignment:
            # RS[i, :] = sum of E rows in block i ; colsum = sum_i P_i @ RS_i
            psRS = ps.tile([8, 32], F32, tag="ps", name="psRS")
            nc.tensor.matmul(psRS[:], lhsT=bdones[:], rhs=E[:],
                             start=True, stop=True)
            RS = wp.tile([8, 32], F32, tag="RS")
            nc.vector.tensor_copy(RS[:], psRS[:])
            psRST = ps.tile([32, 8], F32, tag="ps", name="psRST")
            nc.tensor.matmul(psRST[:], lhsT=RS[:], rhs=ident[0:8, 0:8],
                             is_transpose=True)
            RST = wp.tile([32, 8], F32, tag="RST")
            nc.vector.tensor_copy(RST[:], psRST[:])
            pscol = ps.tile([32, 1], F32, tag="ps", name="pscol")
            for i in range(8):
                nc.tensor.matmul(pscol[:], lhsT=pcat[:, i * 32:(i + 1) * 32],
                                 rhs=RST[:, i:i + 1],
                                 start=(i == 0), stop=(i == 7))
            collse = wp.tile([32, 1], F32, tag="collse")
            nc.scalar.activation(collse[:], pscol[:], ACT.Ln)

            # loss = -(1/64) * [ sum_b (2 diag - rowlse) - sum_c collse ]
            lb = wp.tile([32, 1], F32, tag="lb")
            nc.vector.scalar_tensor_tensor(
                out=lb[:], in0=diag[:], scalar=2.0, in1=rowlse[:],
                op0=ALU.mult, op1=ALU.subtract)
            pl = ps.tile([1, 1], F32, tag="ps", name="pl")
            nc.tensor.matmul(pl[:], lhsT=lb[:], rhs=nones[:, 0:1],
                             start=True, stop=False)
            nc.tensor.matmul(pl[:], lhsT=collse[:], rhs=nones[:, 1:2],
                             start=False, stop=True)
            res = wp.tile([1, 1], F32, tag="res")
            nc.scalar.mul(res[:], pl[:], -1.0 / 64.0)
            od = nc.sync.dma_start(out=out[:], in_=res[:])

            # ---- semaphore hygiene for repeated executions ----------------
            # (lsem is never waited on with absolute thresholds, so it can
            # stay uncleared; the sim race detector also cannot see through
            # its DMA-completion updates.)
            for s in rsems:
                c = nc.gpsimd.sem_clear(s)
                add_dep_helper(c.ins, od.ins, sync=False,
                               reason="clear at end")
                for a in anchors:
                    add_dep_helper(c.ins, a.ins, sync=False,
                                   reason="clear after waits")

    # ---- post-scheduling: attach remote-semaphore waits -------------------
    # (2 increments per arriving send, 2 sends per peer)
    for d in range(1, N_CORES):
        anchors[d - 1].wait_op(rsems[d - 1], 4, "sem-ge")

    nc.finalize()
    return nc


_NC_CACHE = None


def kernel(audio_feats: np.ndarray, visual_feats: np.ndarray) -> np.ndarray:
    import ml_dtypes
    from concourse.bass_utils import run_bass_kernel_spmd

    global _NC_CACHE
    if _NC_CACHE is None:
        _NC_CACHE = build()
    nc = _NC_CACHE

    audio = np.asarray(audio_feats, dtype=np.float32).astype(
        ml_dtypes.bfloat16)
    visual = np.asarray(visual_feats, dtype=np.float32).astype(
        ml_dtypes.bfloat16)
    in_maps = []
    for i in range(N_CORES):
        in_maps.append({
            "audio": np.ascontiguousarray(
                audio[i * BL:(i + 1) * BL].reshape(AROWS, D)),
            "visual": np.ascontiguousarray(
                visual[i * BL:(i + 1) * BL].reshape(VROWS, D)),
        })
    res = run_bass_kernel_spmd(nc, in_maps, core_ids=list(range(N_CORES)))
    val = res.results[0]["out"][0, 0]
    return np.asarray(val, dtype=np.float32)


if __name__ == "__main__":
    rng = np.random.default_rng(0)
    a = rng.standard_normal((B, NA, D)).astype(np.float32)
    v = rng.standard_normal((B, NV, D)).astype(np.float32)
    print(kernel(a, v))
